# revision 17
# baseline (speedup 1.0000x reference)
"""Trainium2 Bass kernel for nn_AttentionModule (S=2048, D=4096, H=32, KV=8, HD=128).

Sharding: tensor-parallel over heads across 8 NeuronCores. Core c owns q-heads
4c..4c+3 and kv-head c (GQA groups stay intact). Each core computes RMSNorm
(norm_w folded into weights on host, rstd computed on device), its QKV
projection shard, RoPE, causal attention for its 4 heads, and a partial output
projection against its 512 columns of wo. The host sums the 8 partial outputs
(the "all-reduce" of the tensor-parallel layout).

All matmuls run as float32r (TF32-like single-pass mode, 1 cycle/row at free
dim >= 256 vs 4 cycles/row for exact fp32).

Layout notes:
 - Everything on-chip is "transposed": hT [d, s], qT/kT/vT [head_dim, s].
   Host pre-transposes hidden and the weight shards so the contraction dim is
   always the partition dim.
 - RoPE: the reference uses interleaved complex pairs (2i, 2i+1). We permute
   the head-dim rows of wq/wk on the host so pairs land at (i, i+64), turning
   RoPE into rotate-half form: q' = q*cos + (P_rot@q)*sin, computed with one
   128x128 signed-permutation matmul + 3 vector ops per tile.
 - Softmax runs in scores-transposed [t, s] layout: denominators via a
   ones-column matmul (reduction over the partition dim), reciprocal on DVE,
   broadcast back over partitions via a K=1 ones-row matmul.
 - Causal masking: full t-chunks below the diagonal need no mask; the 4
   diagonal chunks per s-block use affine_select on GPSIMD
   (iota = j - p - 128r >= 0).
 - All ACT activations (Exp, Ln, Copy) are kept inside one table set
   (natural_log_exp_and_others) to avoid ~1.3us table reloads; the Bacc
   subclass below reorders the candidate tables so that set wins.
"""
import sys

sys.path.insert(0, "/opt/trn_rl_repo")

import math
from contextlib import ExitStack

import numpy as np

import bass_rust as _bass_rust
import concourse.bacc as bacc
import concourse.mybir as mybir
import concourse.tile as tile
from concourse.bass_utils import run_bass_kernel_spmd
from concourse.hw_specs import get_activation_tables

F32R = mybir.dt.float32r
F32 = mybir.dt.float32
ALU = mybir.AluOpType
ACTF = mybir.ActivationFunctionType

S, D, H, KV, HD = 2048, 4096, 32, 8, 128
NCORES = 8
QH = H // NCORES          # 4 q heads per core
QI = QH * HD              # 512 local q dims
DC = D // 128             # 32 contraction chunks
SB = 512                  # s-block width
NSB = S // SB             # 4 s-blocks
NTC = S // 128            # 16 t-chunks
EPS = 1e-6
THETA = 50000.0
SM_SCALE = 1.0 / math.sqrt(HD)

LAST_EXEC_NS = None
LAST_RESULT = None
_CACHE = {}

# pipeline-depth knobs (tuned via timeline sim)
KNOBS = dict(hb_bufs=8, sq_act=True, t12_bufs=1, expp_bufs=3, qtmp_bufs=3,
             sc_bufs=2, wkv_bufs=3, sqp_bufs=2, hb_dc=2, interleave=True,
             mask_dve=True)


class _Bacc(bacc.Bacc):
    """Bacc with activation tables reordered so the one set containing
    Exp+Ln+Copy+Square is preferred — avoids per-call ACT table reloads."""

    def insert_act_table_loads(self):
        has_activation = any(
            isinstance(i, mybir.InstActivation)
            for b in self.main_func.blocks
            for i in b.instructions
        )
        if not has_activation:
            return
        tables = list(get_activation_tables(self.m.arch).items())
        tables.sort(key=lambda kv: 0 if kv[0] == "natural_log_exp_and_others" else 1)
        _bass_rust.insert_act_table_loads(self, tables)


def _build(skip_compile=False):
    nc = bacc.Bacc("TRN2", target_bir_lowering=False, debug=False)

    hT_d = nc.dram_tensor("hT", [D, S], F32R, kind="ExternalInput")
    wqT_d = nc.dram_tensor("wqT", [D, QI], F32R, kind="ExternalInput")
    wkT_d = nc.dram_tensor("wkT", [D, HD], F32R, kind="ExternalInput")
    wvT_d = nc.dram_tensor("wvT", [D, HD], F32R, kind="ExternalInput")
    woT_d = nc.dram_tensor("woT", [QI, D], F32R, kind="ExternalInput")
    cos_d = nc.dram_tensor("cosT", [128, S], F32R, kind="ExternalInput")
    sin_d = nc.dram_tensor("sinT", [128, S], F32R, kind="ExternalInput")
    prot_d = nc.dram_tensor("protT", [128, 128], F32R, kind="ExternalInput")
    ident_d = nc.dram_tensor("ident", [128, 128], F32R, kind="ExternalInput")
    onec_d = nc.dram_tensor("ones_col", [128, 1], F32R, kind="ExternalInput")
    oner_d = nc.dram_tensor("ones_row", [1, 128], F32R, kind="ExternalInput")
    mask_d = nc.dram_tensor("maskT", [128, 4 * SB], F32R, kind="ExternalInput")
    out_d = nc.dram_tensor("outp", [S, D], F32, kind="ExternalOutput")
    if KNOBS.get("debug_dumps", False):
        dbg_q = nc.dram_tensor("dbg_q", [128, QH, S], F32, kind="ExternalOutput")
        dbg_k = nc.dram_tensor("dbg_k", [128, S], F32, kind="ExternalOutput")
        dbg_vn = nc.dram_tensor("dbg_vn", [128, NTC, HD], F32, kind="ExternalOutput")
        dbg_at = nc.dram_tensor("dbg_at", [128, QH, S], F32, kind="ExternalOutput")
        dbg_rb = nc.dram_tensor("dbg_rb", [128, NSB, SB], F32, kind="ExternalOutput")
        dbg_sq = nc.dram_tensor("dbg_sq", [128, NSB, SB], F32, kind="ExternalOutput")

    hT3 = hT_d.rearrange("(o p) s -> p o s", p=128)      # [128, 32, 2048]
    wqT3 = wqT_d.rearrange("(o p) i -> p o i", p=128)    # [128, 32, 512]
    wkT3 = wkT_d.rearrange("(o p) e -> p o e", p=128)    # [128, 32, 128]
    wvT3 = wvT_d.rearrange("(o p) e -> p o e", p=128)
    woT3 = woT_d.rearrange("(g p) j -> p g j", p=128)    # [128, 4, 4096]
    out4 = out_d.rearrange("(g p) j -> p g j", p=128)    # [128, 16, 4096]

    HB_DC = KNOBS.get("hb_dc", 2)  # hT chunks per DMA

    with tile.TileContext(nc) as tc:
        with ExitStack() as root:
            consts = root.enter_context(tc.tile_pool(name="consts", bufs=1))
            persist = root.enter_context(tc.tile_pool(name="persist", bufs=1))

            onec_t = consts.tile([128, 1], F32R, tag="onec")
            nc.sync.dma_start(out=onec_t, in_=onec_d[:, :])
            oner_t = consts.tile([1, 128], F32R, tag="oner")
            nc.sync.dma_start(out=oner_t, in_=oner_d[:, :])
            eps_t = consts.tile([1, 1], F32, tag="eps")
            nc.vector.memset(eps_t, EPS)

            qT_all = persist.tile([128, QH, S], F32R, tag="qT")
            kT_all = persist.tile([128, S], F32R, tag="kT")
            v_nat = persist.tile([128, NTC, HD], F32R, tag="vn")

            # ------------- Phase 1: QKV projections + rstd + RoPE -------------
            with ExitStack() as ph1:
                c1 = ph1.enter_context(tc.tile_pool(name="c1", bufs=1))
                cos_t = c1.tile([128, S], F32R, tag="cos")
                sin_t = c1.tile([128, S], F32R, tag="sin")
                prot_t = c1.tile([128, 128], F32R, tag="prot")
                ident_t = c1.tile([128, 128], F32R, tag="ident")
                c1_loaded = [False]

                wqp = ph1.enter_context(tc.tile_pool(name="wqp", bufs=1))
                wq_t = wqp.tile([128, DC, QI], F32R, tag="wqr")
                wkvp = ph1.enter_context(tc.tile_pool(name="wkvp", bufs=KNOBS["wkv_bufs"]))
                hb = ph1.enter_context(tc.tile_pool(name="hb", bufs=KNOBS["hb_bufs"]))
                sqp = ph1.enter_context(tc.tile_pool(name="sqp", bufs=KNOBS["sqp_bufs"]))
                scr = ph1.enter_context(tc.tile_pool(name="scr", bufs=2))
                acc_ps = ph1.enter_context(
                    tc.tile_pool(name="acc_ps", bufs=1, space="PSUM")
                )
                misc_ps = ph1.enter_context(
                    tc.tile_pool(name="misc_ps", bufs=2, space="PSUM")
                )

                for sb in range(NSB):
                    ssl = slice(SB * sb, SB * (sb + 1))
                    q_ps = [
                        acc_ps.tile([128, SB], F32, tag=f"q{i}", name=f"q_ps{i}")
                        for i in range(QH)
                    ]
                    k_ps = acc_ps.tile([128, SB], F32, tag="k")
                    v_ps = acc_ps.tile([128, SB], F32, tag="v")
                    sqacc = scr.tile([128, SB], F32, tag="sqacc", bufs=2)
                    sqr = scr.tile([128, SB], F32R, tag="sqr", bufs=1)
                    for hc in range(DC // HB_DC):
                        ht2 = hb.tile([128, HB_DC, SB], F32R, tag="h")
                        nc.sync.dma_start(out=ht2, in_=hT3[:, HB_DC*hc:HB_DC*(hc+1), ssl])
                        wkc = wkvp.tile([128, HB_DC, HD], F32R, tag="wk2")
                        nc.sync.dma_start(
                            out=wkc, in_=wkT3[:, HB_DC*hc:HB_DC*(hc+1), :])
                        wvc = wkvp.tile([128, HB_DC, HD], F32R, tag="wv2")
                        nc.sync.dma_start(
                            out=wvc, in_=wvT3[:, HB_DC*hc:HB_DC*(hc+1), :])
                        for j in range(HB_DC):
                            dc = HB_DC * hc + j
                            ht = ht2[:, j, :]
                            if sb == 0:
                                nc.sync.dma_start(out=wq_t[:, dc, :], in_=wqT3[:, dc, :])
                            wqc = wq_t[:, dc, :]
                            if sb == 0 and dc == 8 and not c1_loaded[0]:
                                nc.sync.dma_start(out=cos_t, in_=cos_d[:, :])
                                nc.sync.dma_start(out=sin_t, in_=sin_d[:, :])
                                nc.sync.dma_start(out=prot_t, in_=prot_d[:, :])
                                nc.sync.dma_start(out=ident_t, in_=ident_d[:, :])
                                c1_loaded[0] = True
                            sq = sqp.tile([128, SB], F32, tag="sq")
                            if KNOBS["sq_act"]:
                                nc.scalar.activation(out=sq, in_=ht, func=ACTF.Square)
                            else:
                                nc.vector.tensor_tensor(sq, ht, ht, ALU.mult)
                            if dc == 0:
                                nc.vector.tensor_copy(out=sqacc, in_=sq)
                            elif dc == DC - 1:
                                nc.vector.tensor_tensor(sqr, sqacc, sq, ALU.add)
                            else:
                                nc.vector.tensor_tensor(sqacc, sqacc, sq, ALU.add)
                            for i in range(QH):
                                nc.tensor.matmul(
                                    q_ps[i],
                                    wqc[:, 128 * i: 128 * (i + 1)],
                                    ht,
                                    start=(dc == 0),
                                    stop=(dc == DC - 1),
                                )
                            nc.tensor.matmul(
                                k_ps, wkc[:, j, :], ht,
                                start=(dc == 0), stop=(dc == DC - 1),
                            )
                            nc.tensor.matmul(
                                v_ps, wvc[:, j, :], ht,
                                start=(dc == 0), stop=(dc == DC - 1),
                            )
                    # rstd row for this s-block (exp(-0.5 ln(ms)) — same ACT set).
                    # PSUM evacuation is plain copies (no rstd dependency) so the
                    # next s-block's accumulation starts immediately; rstd is
                    # folded into per-block cos/sin tables instead.
                    ms_ps = misc_ps.tile([1, SB], F32, tag="misc", name="ms_ps")
                    nc.tensor.matmul(ms_ps, onec_t, sqr, start=True, stop=True)
                    lnt = scr.tile([1, SB], F32, tag="lnt", bufs=1)
                    nc.scalar.activation(
                        out=lnt, in_=ms_ps, func=ACTF.Sqrt, scale=1.0 / D, bias=eps_t
                    )
                    rstd = scr.tile([1, SB], F32R, tag="rstd", bufs=1)
                    with nc.allow_low_precision(reason="rstd row fp32r"):
                        nc.vector.reciprocal(out=rstd, in_=lnt.bitcast(F32R))
                    rb_ps = misc_ps.tile([128, SB], F32, tag="misc", name="rb_ps")
                    nc.tensor.matmul(rb_ps, oner_t, rstd, start=True, stop=True)
                    rb_sb = scr.tile([128, SB], F32R, tag="rb_sb", bufs=2)
                    nc.vector.tensor_copy(out=rb_sb, in_=rb_ps.bitcast(F32R))
                    if KNOBS.get("debug_dumps", False):
                        nc.sync.dma_start(out=dbg_rb[:, sb, :], in_=rb_sb.bitcast(F32))
                        nc.sync.dma_start(out=dbg_sq[:, sb, :], in_=sqr.bitcast(F32))
                    cosrb = scr.tile([128, SB], F32R, tag="cosrb", bufs=2)
                    nc.vector.tensor_tensor(cosrb, cos_t[:, ssl], rb_sb, ALU.mult)
                    sinrb = scr.tile([128, SB], F32R, tag="sinrb", bufs=2)
                    nc.vector.tensor_tensor(sinrb, sin_t[:, ssl], rb_sb, ALU.mult)

                    # q + rope (scale folded into cosrb/sinrb) -> qT_all
                    for i in range(QH):
                        qtmp = scr.tile([128, SB], F32R, tag="qtmp", bufs=KNOBS["qtmp_bufs"])
                        nc.vector.tensor_copy(out=qtmp, in_=q_ps[i].bitcast(F32R))
                        rot_ps = misc_ps.tile([128, SB], F32, tag="misc",
                                              name=f"rot_q{i}")
                        nc.tensor.matmul(rot_ps, prot_t, qtmp, start=True, stop=True)
                        t1 = scr.tile([128, SB], F32R, tag="t1", bufs=KNOBS["t12_bufs"])
                        nc.vector.tensor_tensor(t1, qtmp, cosrb, ALU.mult)
                        t2 = scr.tile([128, SB], F32R, tag="t2", bufs=KNOBS["t12_bufs"])
                        nc.vector.tensor_tensor(
                            t2, rot_ps.bitcast(F32R), sinrb, ALU.mult
                        )
                        nc.vector.tensor_tensor(qT_all[:, i, ssl], t1, t2, ALU.add)
                    # k + rope -> kT_all
                    ktmp = scr.tile([128, SB], F32R, tag="qtmp", bufs=KNOBS["qtmp_bufs"], name="ktmp")
                    nc.vector.tensor_copy(out=ktmp, in_=k_ps.bitcast(F32R))
                    rot_ps = misc_ps.tile([128, SB], F32, tag="misc", name="rot_k")
                    nc.tensor.matmul(rot_ps, prot_t, ktmp, start=True, stop=True)
                    t1 = scr.tile([128, SB], F32R, tag="t1", bufs=KNOBS["t12_bufs"], name="t1k")
                    nc.vector.tensor_tensor(t1, ktmp, cosrb, ALU.mult)
                    t2 = scr.tile([128, SB], F32R, tag="t2", bufs=KNOBS["t12_bufs"], name="t2k")
                    nc.vector.tensor_tensor(
                        t2, rot_ps.bitcast(F32R), sinrb, ALU.mult
                    )
                    nc.vector.tensor_tensor(kT_all[:, ssl], t1, t2, ALU.add)
                    # v: evacuate, scale by rstd, transpose to v_nat
                    vtmp = scr.tile([128, SB], F32R, tag="qtmp", bufs=KNOBS["qtmp_bufs"], name="vtmp")
                    nc.vector.tensor_copy(out=vtmp, in_=v_ps.bitcast(F32R))
                    vsc = scr.tile([128, SB], F32R, tag="vsc", bufs=2)
                    nc.vector.tensor_tensor(vsc, vtmp, rb_sb, ALU.mult)
                    for j in range(SB // 128):
                        tcx = (SB // 128) * sb + j
                        vtr_ps = misc_ps.tile([128, 128], F32R, tag="misc",
                                              name=f"vtr{tcx}")
                        nc.tensor.transpose(
                            vtr_ps, vsc[:, 128 * j: 128 * (j + 1)], ident_t
                        )
                        nc.vector.tensor_copy(out=v_nat[:, tcx, :], in_=vtr_ps)

            # attnT allocated only now (frees phase-1 SBUF for resident wq)
            persist2 = root.enter_context(tc.tile_pool(name="persist2", bufs=1))
            attnT = persist2.tile([128, QH, S], F32R, tag="attnT")
            mask_t = persist2.tile([128, 4, SB], F32R, tag="mask")
            nc.sync.dma_start(out=mask_t, in_=mask_d.rearrange("p (r s) -> p r s", s=SB))

            # phase-4 pools allocated first so they get PSUM banks / SBUF
            # disjoint from phase 3 (enables clean overlap)
            o_ps_p = root.enter_context(tc.tile_pool(name="o_ps", bufs=2, space="PSUM"))
            outb = root.enter_context(tc.tile_pool(name="outb", bufs=2))
            wop = root.enter_context(tc.tile_pool(name="wop", bufs=2))

            # ------------- Phase 3+4 interleaved ------------------------------
            sc_ps_p = root.enter_context(
                tc.tile_pool(name="sc_ps", bufs=KNOBS["sc_bufs"], space="PSUM")
            )
            att_ps_p = root.enter_context(
                tc.tile_pool(name="att_ps", bufs=1, space="PSUM")
            )
            sum_ps_p = root.enter_context(
                tc.tile_pool(name="sum_ps", bufs=1, space="PSUM")
            )
            expp = root.enter_context(tc.tile_pool(name="expp", bufs=KNOBS["expp_bufs"]))
            scr3 = root.enter_context(tc.tile_pool(name="scr3", bufs=2))

            def emit_attention(sb):
                for h in range(QH):
                    ssl = slice(SB * sb, SB * (sb + 1))
                    n_tc = (SB // 128) * (sb + 1)
                    att_ps = att_ps_p.tile([128, SB], F32, tag="att",
                                           name=f"att{h}_{sb}")
                    sum_ps = sum_ps_p.tile([1, SB], F32, tag="sumrc",
                                           name=f"sum{h}_{sb}")
                    for tp in range(n_tc // 2):
                        # paired scores tiles -> one 1024-wide exp
                        sc_ps = sc_ps_p.tile([128, 2, SB], F32, tag="sc",
                                             name=f"sc{h}_{sb}_{tp}")
                        e_pair = expp.tile([128, 2, SB], F32R, tag="e",
                                           name=f"e{h}_{sb}_{tp}")
                        for u in range(2):
                            tcx = 2 * tp + u
                            nc.tensor.matmul(
                                sc_ps[:, u, :],
                                kT_all[:, 128 * tcx: 128 * (tcx + 1)],
                                qT_all[:, h, ssl],
                                start=True, stop=True,
                            )
                        nc.scalar.activation(
                            out=e_pair, in_=sc_ps, func=ACTF.Exp, scale=SM_SCALE
                        )
                        for u in range(2):
                            tcx = 2 * tp + u
                            e_sb = e_pair[:, u, :]
                            r = tcx - (SB // 128) * sb
                            if r >= 0:
                                # diagonal chunk: zero where t > s
                                if KNOBS.get("mask_dve", True):
                                    nc.vector.tensor_tensor(
                                        e_sb, e_sb, mask_t[:, r, :], ALU.mult
                                    )
                                else:
                                    nc.gpsimd.affine_select(
                                        e_sb, e_sb,
                                        pattern=[[1, SB]],
                                        compare_op=ALU.is_ge,
                                        fill=0.0,
                                        base=-(128 * r),
                                        channel_multiplier=-1,
                                    )
                            nc.tensor.matmul(
                                att_ps, v_nat[:, tcx, :], e_sb,
                                start=(tcx == 0), stop=(tcx == n_tc - 1),
                            )
                            nc.tensor.matmul(
                                sum_ps, onec_t, e_sb,
                                start=(tcx == 0), stop=(tcx == n_tc - 1),
                            )
                    # reciprocal of denominators on DVE
                    rcv = scr3.tile([1, SB], F32R, tag="rcv", bufs=2,
                                    name=f"rcv{h}_{sb}")
                    with nc.allow_low_precision(reason="softmax recip row"):
                        nc.vector.reciprocal(out=rcv, in_=sum_ps.bitcast(F32R))
                    rc_ps = sum_ps_p.tile([128, SB], F32, tag="sumrc",
                                          name=f"rc{h}_{sb}")
                    nc.tensor.matmul(rc_ps, oner_t, rcv, start=True, stop=True)
                    rc_sb = scr3.tile([128, SB], F32R, tag="rcsb", bufs=2,
                                      name=f"rcsb{h}_{sb}")
                    nc.vector.tensor_copy(out=rc_sb, in_=rc_ps.bitcast(F32R))
                    nc.vector.tensor_tensor(
                        attnT[:, h, ssl], att_ps.bitcast(F32R), rc_sb, ALU.mult
                    )

            def emit_outproj(g):
                for jt in range(D // SB):
                    jsl = slice(SB * jt, SB * (jt + 1))
                    woc = wop.tile([128, QH, SB], F32R, tag="wo",
                                   name=f"wo{jt}_{g}")
                    nc.sync.dma_start(out=woc, in_=woT3[:, :, jsl])
                    o_big = outb.tile([128, 8, SB], F32, tag="obig",
                                      name=f"ob{jt}_{g}")
                    for si in range(8):
                        sc = 8 * g + si
                        o_ps = o_ps_p.tile([128, SB], F32, tag="o",
                                           name=f"o{jt}_{sc}")
                        for h in range(QH):
                            nc.tensor.matmul(
                                o_ps,
                                attnT[:, h, 128 * sc: 128 * (sc + 1)],
                                woc[:, h, :],
                                start=(h == 0), stop=(h == QH - 1),
                            )
                        if si % 2 == 0:
                            nc.vector.tensor_copy(out=o_big[:, si, :], in_=o_ps)
                        else:
                            nc.scalar.copy(out=o_big[:, si, :], in_=o_ps)
                    nc.sync.dma_start(
                        out=out4[:, 8 * g: 8 * (g + 1), jsl], in_=o_big
                    )

            if KNOBS.get("debug_dumps", False):
                nc.sync.dma_start(out=dbg_q[:, :, :], in_=qT_all.bitcast(F32))
                nc.sync.dma_start(out=dbg_k[:, :], in_=kT_all.bitcast(F32))
                nc.sync.dma_start(out=dbg_vn[:, :, :], in_=v_nat.bitcast(F32))
            if KNOBS.get("interleave", True):
                emit_attention(0)
                emit_attention(1)
                emit_outproj(0)   # sc 0..7 only needs attnT of sb 0-1
                emit_attention(2)
                emit_attention(3)
                emit_outproj(1)
                if KNOBS.get("debug_dumps", False):
                    nc.sync.dma_start(out=dbg_at[:, :, :], in_=attnT.bitcast(F32))
            else:
                for _sb in range(NSB):
                    emit_attention(_sb)
                emit_outproj(0)
                emit_outproj(1)

    if not skip_compile:
        nc.compile()
    return nc


def _host_prep(inputs):
    """Build per-core input maps (shard + transpose + fold norm_w + rope-perm)."""
    hidden = np.ascontiguousarray(np.asarray(inputs["hidden"], dtype=np.float32))
    norm_w = np.asarray(inputs["norm_w"], dtype=np.float32)
    wq = np.asarray(inputs["wq"], dtype=np.float32)
    wk = np.asarray(inputs["wk"], dtype=np.float32)
    wv = np.asarray(inputs["wv"], dtype=np.float32)
    wo = np.asarray(inputs["wo"], dtype=np.float32)

    perm = np.concatenate([np.arange(0, HD, 2), np.arange(1, HD, 2)])
    # RoPE tables exactly as the reference builds them
    freqs = 1.0 / THETA ** (np.arange(0, HD, 2)[: HD // 2].astype(np.float32) / HD)
    ang = np.outer(np.arange(S), freqs).astype(np.float32)   # [S, 64]
    cosT = np.ascontiguousarray(
        np.concatenate([np.cos(ang).T, np.cos(ang).T], axis=0).astype(np.float32)
    )
    sinT = np.ascontiguousarray(
        np.concatenate([np.sin(ang).T, np.sin(ang).T], axis=0).astype(np.float32)
    )
    Pr = np.zeros((HD, HD), np.float32)
    Pr[np.arange(64), np.arange(64) + 64] = -1.0
    Pr[np.arange(64) + 64, np.arange(64)] = 1.0
    protT = np.ascontiguousarray(Pr.T)

    hT = np.ascontiguousarray(hidden.T)
    ident = np.eye(128, dtype=np.float32)
    # diagonal causal masks: maskT[p, r*512 + c] = 1 if 128*r + p <= c else 0
    p_i = np.arange(128)[:, None]
    c_i = np.arange(SB)[None, :]
    maskT = np.concatenate(
        [(128 * r + p_i <= c_i).astype(np.float32) for r in range(4)], axis=1
    )
    maskT = np.ascontiguousarray(maskT)
    ones_col = np.ones((128, 1), np.float32)
    ones_row = np.ones((1, 128), np.float32)

    in_maps = []
    for c in range(NCORES):
        wq_c = wq[QI * c: QI * (c + 1)].reshape(QH, HD, D)[:, perm, :].reshape(QI, D)
        wqT = np.ascontiguousarray((wq_c * norm_w[None, :]).T)
        wk_c = wk[HD * c: HD * (c + 1)][perm, :]
        wkT = np.ascontiguousarray((wk_c * norm_w[None, :]).T)
        wv_c = wv[HD * c: HD * (c + 1)]
        wvT = np.ascontiguousarray((wv_c * norm_w[None, :]).T)
        woT = np.ascontiguousarray(wo[:, QI * c: QI * (c + 1)].T)
        in_maps.append({
            "hT": hT, "wqT": wqT, "wkT": wkT, "wvT": wvT, "woT": woT,
            "cosT": cosT, "sinT": sinT, "protT": protT, "ident": ident,
            "ones_col": ones_col, "ones_row": ones_row, "maskT": maskT,
        })
    return in_maps


def kernel(**inputs) -> np.ndarray:
    global LAST_EXEC_NS, LAST_RESULT
    if "nc" not in _CACHE:
        _CACHE["nc"] = _build()
    nc = _CACHE["nc"]
    in_maps = _host_prep(inputs)
    res = run_bass_kernel_spmd(nc, in_maps, core_ids=list(range(NCORES)))
    LAST_RESULT = res
    LAST_EXEC_NS = res.exec_time_ns
    out = res.results[0]["outp"].astype(np.float32).copy()
    for c in range(1, NCORES):
        out += res.results[c]["outp"]
    return out
